# revision 17
# baseline (speedup 1.0000x reference)
"""Window-routed sparse attention on 8 TRN2 NeuronCores.

Sharding: 64 windows x 8 cores = 8 windows/core (embarrassingly parallel).
Host precomputes the tiny routing path (region means, a_r [64,64]) and the
window-mixed q_m/k_m in fp32; each core runs the heavy windowed attention
relu(q_m k_m^T) v for its 8 windows:

  - QK^T in bf16 (f32 PSUM), software-pipelined so the PE never waits
  - relu runs on Scalar (even chunks) and Vector (odd chunks) engines,
    emitting scaled fp8e4 attention weights
  - attn @ v in fp8e4 with DoubleRow perf mode (2x matmul throughput),
    accumulating 256 contraction rows per step in f32 PSUM
"""

import sys

sys.path.insert(0, "/opt/trn_rl_repo")

import numpy as np
import ml_dtypes

C = 64          # channels
NW = 64         # windows (8x8 grid of 32x32 patches on 256x256)
T = 1024        # tokens per window (32*32)
NCORES = 8
WPC = NW // NCORES  # windows per core
BF16 = ml_dtypes.bfloat16

_CACHE = {}


def _build_program(s_a):
    import concourse.mybir as mybir
    from concourse import bacc
    from concourse.tile import TileContext

    bf16 = mybir.dt.bfloat16
    f32 = mybir.dt.float32
    f8 = mybir.dt.float8e4

    nc = bacc.Bacc(None, target_bir_lowering=False)
    # c-major [c, i, t] for q_m/k_m; v pre-packed for DoubleRow:
    # v8[p, i, kk, j, c] = v[i, 256*kk + 128*j + p, c] (scaled to fp8)
    qm_d = nc.declare_dram_parameter("qm", [C, WPC, T], bf16, isOutput=False)
    km_d = nc.declare_dram_parameter("km", [C, WPC, T], bf16, isOutput=False)
    v_d = nc.declare_dram_parameter("v", [128, WPC, 4, 2, C], f8, isOutput=False)
    o_d = nc.declare_dram_parameter("o", [C, WPC, T], bf16, isOutput=True)

    with TileContext(nc) as tc:
        with (
            tc.tile_pool(name="in", bufs=1) as in_pool,
            tc.tile_pool(name="at", bufs=3) as a_pool,
            tc.tile_pool(name="pa", bufs=3, space="PSUM") as pa_pool,
            tc.tile_pool(name="po", bufs=1, space="PSUM") as po_pool,
        ):
            # persistent SBUF tiles, one set per window so each window's
            # compute waits only on its own three DMAs
            qm_w, km_w, v_w, o_w = [], [], [], []
            for i in range(WPC):
                qm_i = in_pool.tile([C, T], bf16, tag=f"qm{i}", name=f"qm{i}")
                km_i = in_pool.tile([C, T], bf16, tag=f"km{i}", name=f"km{i}")
                v_i = in_pool.tile([128, 4, 2, C], f8, tag=f"v{i}", name=f"v{i}")
                o_i = in_pool.tile([C, T], bf16, tag=f"o{i}", name=f"o{i}")
                # stripe each tensor's DMA across 4 queues (partition strips)
                # so a single queue's ~145ns/descriptor rate isn't the gate
                for p in range(4):
                    nc.sync.dma_start(
                        out=qm_i[p * 16:(p + 1) * 16], in_=qm_d[p * 16:(p + 1) * 16, i]
                    )
                    nc.sync.dma_start(
                        out=km_i[p * 16:(p + 1) * 16], in_=km_d[p * 16:(p + 1) * 16, i]
                    )
                    nc.sync.dma_start(
                        out=v_i[p * 32:(p + 1) * 32], in_=v_d[p * 32:(p + 1) * 32, i]
                    )
                qm_w.append(qm_i); km_w.append(km_i)
                v_w.append(v_i); o_w.append(o_i)

            for i in range(WPC):
                # pipelined per window: QK/relu for chunk pair kk+1 issue
                # before the fp8 DoubleRow AV for pair kk
                ps_o = po_pool.tile([C, T], f32, tag="pso")
                at_live = {}

                def emit_pair(kk):
                    # h-major fp8 attn tile: at8[:, h, j, :] so the DoubleRow
                    # rhs slice at8[:, h] is contiguous per partition, and
                    # relu runs at quarter granularity for finer pipelining
                    at8 = a_pool.tile([128, 2, 2, 512], f8, tag="attn", name="attn")
                    for j in range(2):
                        k = 2 * kk + j
                        ps_a = pa_pool.tile([128, T], f32, tag="psa", name="psa")
                        for h in range(2):
                            nc.tensor.matmul(
                                out=ps_a[:, h * 512:(h + 1) * 512],
                                lhsT=km_w[i][:, k * 128:(k + 1) * 128],
                                rhs=qm_w[i][:, h * 512:(h + 1) * 512],
                                start=True,
                                stop=True,
                            )
                        for h in range(2):
                            if j == 0:
                                nc.scalar.activation(
                                    out=at8[:, h, 0, :],
                                    in_=ps_a[:, h * 512:(h + 1) * 512],
                                    func=mybir.ActivationFunctionType.Relu,
                                    scale=float(s_a),
                                )
                            else:
                                nc.vector.tensor_scalar(
                                    out=at8[:, h, 1, :],
                                    in0=ps_a[:, h * 512:(h + 1) * 512],
                                    scalar1=float(s_a),
                                    scalar2=0.0,
                                    op0=mybir.AluOpType.mult,
                                    op1=mybir.AluOpType.max,
                                )
                    at_live[kk] = at8

                emit_pair(0)
                for kk in range(4):
                    if kk + 1 < 4:
                        emit_pair(kk + 1)
                    at8 = at_live.pop(kk)
                    for h in range(2):
                        nc.tensor.matmul(
                            out=ps_o[:, h * 512:(h + 1) * 512],
                            lhsT=v_w[i][:, kk],
                            rhs=at8[:, h],
                            perf_mode=mybir.MatmulPerfMode.DoubleRow,
                            start=(kk == 0),
                            stop=(kk == 3),
                        )
                # split PSUM->SBUF output copy across both free engines
                nc.scalar.activation(
                    out=o_w[i][:, 0:512],
                    in_=ps_o[:, 0:512],
                    func=mybir.ActivationFunctionType.Copy,
                    scale=1.0,
                )
                nc.vector.tensor_copy(out=o_w[i][:, 512:1024], in_=ps_o[:, 512:1024])
                nc.sync.dma_start(out=o_d[:, i], in_=o_w[i])

    nc.finalize()
    return nc


def kernel(x, W, bias, _trace=False):
    global LAST_RESULT
    from concourse.bass_utils import run_bass_kernel_spmd
    import concourse.mybir as mybir

    E4 = mybir.dt.np(mybir.dt.float8e4)

    x = np.asarray(x, dtype=np.float32)
    W = np.asarray(W, dtype=np.float32)
    bias = np.asarray(bias, dtype=np.float32)

    # ---- host prep: windows, qkv, routing, mixing (tiny vs attention) ----
    xw = (
        x.reshape(C, 8, 32, 8, 32)
        .transpose(1, 3, 2, 4, 0)
        .reshape(NW, T, C)
    )
    qkv = xw @ W.T + bias  # [nw, T, 3c]
    q, k, v = qkv[..., :C], qkv[..., C:2 * C], qkv[..., 2 * C:]
    q_r = q.mean(axis=1)  # [nw, c]
    k_r = k.mean(axis=1)
    a_r = np.maximum(q_r @ k_r.T, 0.0)  # [nw, nw]
    k_m = np.tensordot(a_r, k, axes=(1, 0))  # [nw, T, c]
    q_m = np.tensordot(a_r, q, axes=(1, 0))

    # fp8 scales: bound attn logits via Cauchy-Schwarz, v by its max
    bound = max(
        np.linalg.norm(q_m[i], axis=-1).max() * np.linalg.norm(k_m[i], axis=-1).max()
        for i in range(NW)
    )
    s_a = 240.0 / float(bound)
    s_v = 240.0 / float(np.abs(v).max())

    key = ("nc", round(float(s_a), 6))
    if key not in _CACHE:
        _CACHE.clear()
        _CACHE[key] = _build_program(s_a)
    nc = _CACHE[key]

    in_maps = []
    for m in range(NCORES):
        s = slice(m * WPC, (m + 1) * WPC)
        # v8[p, i, kk, j, c] = v[i, 256*kk + 128*j + p, c] * s_v
        v8 = (v[s].reshape(WPC, 4, 2, 128, C) * s_v).astype(E4).transpose(3, 0, 1, 2, 4)
        in_maps.append({
            "qm": np.ascontiguousarray(q_m[s].transpose(2, 0, 1)).astype(BF16),
            "km": np.ascontiguousarray(k_m[s].transpose(2, 0, 1)).astype(BF16),
            "v": np.ascontiguousarray(v8),
        })

    res = run_bass_kernel_spmd(nc, in_maps, list(range(NCORES)), trace=_trace)
    LAST_RESULT = res
    inv = 1.0 / (s_a * s_v)
    outs = [
        res.results[m]["o"].astype(np.float32).reshape(C, WPC, T) * inv
        for m in range(NCORES)
    ]
    o_cm = np.concatenate(outs, axis=1)  # [c, nw, T]

    # fold back: [c, jh, jw, th, tw] -> [1, c, 256, 256]
    o_img = (
        o_cm.reshape(C, 8, 8, 32, 32)
        .transpose(0, 1, 3, 2, 4)
        .reshape(1, C, 256, 256)
    )
    return o_img.astype(np.float32)


LAST_RESULT = None  # BassKernelResults from the most recent run (for test.py)


# revision 19
# speedup vs baseline: 1.0624x; 1.0624x over previous
"""Window-routed sparse attention on 8 TRN2 NeuronCores.

Sharding: 64 windows x 8 cores = 8 windows/core (embarrassingly parallel).
Host precomputes the tiny routing path (region means, a_r [64,64]) and the
window-mixed q_m/k_m in fp32; each core runs the heavy windowed attention
relu(q_m k_m^T) v for its 8 windows:

  - QK^T in bf16 (f32 PSUM), software-pipelined so the PE never waits
  - relu runs on Scalar (even chunks) and Vector (odd chunks) engines,
    emitting scaled fp8e4 attention weights
  - attn @ v in fp8e4 with DoubleRow perf mode (2x matmul throughput),
    accumulating 256 contraction rows per step in f32 PSUM
"""

import sys

sys.path.insert(0, "/opt/trn_rl_repo")

import numpy as np
import ml_dtypes

C = 64          # channels
NW = 64         # windows (8x8 grid of 32x32 patches on 256x256)
T = 1024        # tokens per window (32*32)
NCORES = 8
WPC = NW // NCORES  # windows per core
BF16 = ml_dtypes.bfloat16

_CACHE = {}


def _build_program(s_a):
    import concourse.mybir as mybir
    from concourse import bacc
    from concourse.tile import TileContext

    bf16 = mybir.dt.bfloat16
    f32 = mybir.dt.float32
    f8 = mybir.dt.float8e4

    nc = bacc.Bacc(None, target_bir_lowering=False)
    # c-major [c, i, t] for q_m/k_m; v pre-packed for DoubleRow:
    # v8[p, i, kk, j, c] = v[i, 256*kk + 128*j + p, c] (scaled to fp8)
    qm_d = nc.declare_dram_parameter("qm", [C, WPC, T], bf16, isOutput=False)
    km_d = nc.declare_dram_parameter("km", [C, WPC, T], bf16, isOutput=False)
    v_d = nc.declare_dram_parameter("v", [128, WPC, 4, 2, C], f8, isOutput=False)
    o_d = nc.declare_dram_parameter("o", [C, WPC, T], bf16, isOutput=True)

    with TileContext(nc) as tc:
        with (
            tc.tile_pool(name="in", bufs=1) as in_pool,
            tc.tile_pool(name="at", bufs=3) as a_pool,
            tc.tile_pool(name="pa", bufs=3, space="PSUM") as pa_pool,
            tc.tile_pool(name="po", bufs=1, space="PSUM") as po_pool,
        ):
            # persistent SBUF tiles, one set per window so each window's
            # compute waits only on its own three DMAs
            qm_w, km_w, v_w, o_w = [], [], [], []
            for i in range(WPC):
                qm_i = in_pool.tile([C, T], bf16, tag=f"qm{i}", name=f"qm{i}")
                km_i = in_pool.tile([C, T], bf16, tag=f"km{i}", name=f"km{i}")
                v_i = in_pool.tile([128, 4, 2, C], f8, tag=f"v{i}", name=f"v{i}")
                o_i = in_pool.tile([C, T], bf16, tag=f"o{i}", name=f"o{i}")
                # inputs issue from the (otherwise idle) GpSimd queue so the
                # SP queue only carries output DMAs; window 0's tensors are
                # split in two so its first matmul can start sooner
                if i == 0:
                    nc.gpsimd.dma_start(out=qm_i[0:32], in_=qm_d[0:32, i])
                    nc.sync.dma_start(out=qm_i[32:64], in_=qm_d[32:64, i])
                    nc.gpsimd.dma_start(out=km_i[0:32], in_=km_d[0:32, i])
                    nc.sync.dma_start(out=km_i[32:64], in_=km_d[32:64, i])
                    nc.gpsimd.dma_start(out=v_i[0:64], in_=v_d[0:64, i])
                    nc.sync.dma_start(out=v_i[64:128], in_=v_d[64:128, i])
                else:
                    nc.gpsimd.dma_start(out=qm_i, in_=qm_d[:, i])
                    nc.gpsimd.dma_start(out=km_i, in_=km_d[:, i])
                    nc.gpsimd.dma_start(out=v_i, in_=v_d[:, i])
                qm_w.append(qm_i); km_w.append(km_i)
                v_w.append(v_i); o_w.append(o_i)

            for i in range(WPC):
                # pipelined per window: QK/relu for chunk pair kk+1 issue
                # before the fp8 DoubleRow AV for pair kk
                ps_o = po_pool.tile([C, T], f32, tag="pso")
                at_live = {}

                def emit_pair(kk):
                    # h-major fp8 attn tile: at8[:, h, j, :] so the DoubleRow
                    # rhs slice at8[:, h] is contiguous per partition, and
                    # relu runs at quarter granularity for finer pipelining
                    at8 = a_pool.tile([128, 2, 2, 512], f8, tag="attn", name="attn")
                    for j in range(2):
                        k = 2 * kk + j
                        ps_a = pa_pool.tile([128, T], f32, tag="psa", name="psa")
                        for h in range(2):
                            nc.tensor.matmul(
                                out=ps_a[:, h * 512:(h + 1) * 512],
                                lhsT=km_w[i][:, k * 128:(k + 1) * 128],
                                rhs=qm_w[i][:, h * 512:(h + 1) * 512],
                                start=True,
                                stop=True,
                            )
                        for h in range(2):
                            if j == 0:
                                nc.scalar.activation(
                                    out=at8[:, h, 0, :],
                                    in_=ps_a[:, h * 512:(h + 1) * 512],
                                    func=mybir.ActivationFunctionType.Relu,
                                    scale=float(s_a),
                                )
                            else:
                                nc.vector.tensor_scalar(
                                    out=at8[:, h, 1, :],
                                    in0=ps_a[:, h * 512:(h + 1) * 512],
                                    scalar1=float(s_a),
                                    scalar2=0.0,
                                    op0=mybir.AluOpType.mult,
                                    op1=mybir.AluOpType.max,
                                )
                    at_live[kk] = at8

                if i == 0:
                    # window 0 runs serialized (no QK lookahead): the tensor
                    # engine idles during each relu, presenting the low-duty
                    # activity pattern that makes the HW governor lift the
                    # PE utilization throttle early in the run
                    for kk in range(4):
                        emit_pair(kk)
                        at8 = at_live.pop(kk)
                        for h in range(2):
                            nc.tensor.matmul(
                                out=ps_o[:, h * 512:(h + 1) * 512],
                                lhsT=v_w[i][:, kk],
                                rhs=at8[:, h],
                                perf_mode=mybir.MatmulPerfMode.DoubleRow,
                                start=(kk == 0),
                                stop=(kk == 3),
                            )
                else:
                    emit_pair(0)
                    for kk in range(4):
                        if kk + 1 < 4:
                            emit_pair(kk + 1)
                        at8 = at_live.pop(kk)
                        for h in range(2):
                            nc.tensor.matmul(
                                out=ps_o[:, h * 512:(h + 1) * 512],
                                lhsT=v_w[i][:, kk],
                                rhs=at8[:, h],
                                perf_mode=mybir.MatmulPerfMode.DoubleRow,
                                start=(kk == 0),
                                stop=(kk == 3),
                            )
                # split PSUM->SBUF output copy across both free engines
                nc.scalar.activation(
                    out=o_w[i][:, 0:512],
                    in_=ps_o[:, 0:512],
                    func=mybir.ActivationFunctionType.Copy,
                    scale=1.0,
                )
                nc.vector.tensor_copy(out=o_w[i][:, 512:1024], in_=ps_o[:, 512:1024])
                nc.sync.dma_start(out=o_d[:, i], in_=o_w[i])

    nc.finalize()
    return nc


def kernel(x, W, bias, _trace=False):
    global LAST_RESULT
    from concourse.bass_utils import run_bass_kernel_spmd
    import concourse.mybir as mybir

    E4 = mybir.dt.np(mybir.dt.float8e4)

    x = np.asarray(x, dtype=np.float32)
    W = np.asarray(W, dtype=np.float32)
    bias = np.asarray(bias, dtype=np.float32)

    # ---- host prep: windows, qkv, routing, mixing (tiny vs attention) ----
    xw = (
        x.reshape(C, 8, 32, 8, 32)
        .transpose(1, 3, 2, 4, 0)
        .reshape(NW, T, C)
    )
    qkv = xw @ W.T + bias  # [nw, T, 3c]
    q, k, v = qkv[..., :C], qkv[..., C:2 * C], qkv[..., 2 * C:]
    q_r = q.mean(axis=1)  # [nw, c]
    k_r = k.mean(axis=1)
    a_r = np.maximum(q_r @ k_r.T, 0.0)  # [nw, nw]
    k_m = np.tensordot(a_r, k, axes=(1, 0))  # [nw, T, c]
    q_m = np.tensordot(a_r, q, axes=(1, 0))

    # fp8 scales: bound attn logits via Cauchy-Schwarz, v by its max
    bound = max(
        np.linalg.norm(q_m[i], axis=-1).max() * np.linalg.norm(k_m[i], axis=-1).max()
        for i in range(NW)
    )
    s_a = 240.0 / float(bound)
    s_v = 240.0 / float(np.abs(v).max())

    key = ("nc", round(float(s_a), 6))
    if key not in _CACHE:
        _CACHE.clear()
        _CACHE[key] = _build_program(s_a)
    nc = _CACHE[key]

    in_maps = []
    for m in range(NCORES):
        s = slice(m * WPC, (m + 1) * WPC)
        # v8[p, i, kk, j, c] = v[i, 256*kk + 128*j + p, c] * s_v
        v8 = (v[s].reshape(WPC, 4, 2, 128, C) * s_v).astype(E4).transpose(3, 0, 1, 2, 4)
        in_maps.append({
            "qm": np.ascontiguousarray(q_m[s].transpose(2, 0, 1)).astype(BF16),
            "km": np.ascontiguousarray(k_m[s].transpose(2, 0, 1)).astype(BF16),
            "v": np.ascontiguousarray(v8),
        })

    res = run_bass_kernel_spmd(nc, in_maps, list(range(NCORES)), trace=_trace)
    LAST_RESULT = res
    inv = 1.0 / (s_a * s_v)
    outs = [
        res.results[m]["o"].astype(np.float32).reshape(C, WPC, T) * inv
        for m in range(NCORES)
    ]
    o_cm = np.concatenate(outs, axis=1)  # [c, nw, T]

    # fold back: [c, jh, jw, th, tw] -> [1, c, 256, 256]
    o_img = (
        o_cm.reshape(C, 8, 8, 32, 32)
        .transpose(0, 1, 3, 2, 4)
        .reshape(1, C, 256, 256)
    )
    return o_img.astype(np.float32)


LAST_RESULT = None  # BassKernelResults from the most recent run (for test.py)


# revision 23
# speedup vs baseline: 1.3892x; 1.3076x over previous
"""Window-routed sparse attention on 8 TRN2 NeuronCores.

Sharding: 64 windows x 8 cores = 8 windows/core (embarrassingly parallel).
Host precomputes the tiny routing path (region means, a_r [64,64]) and the
window-mixed q_m/k_m in fp32; each core runs the heavy windowed attention
relu(q_m k_m^T) v for its 8 windows:

  - QK^T in bf16 (f32 PSUM), software-pipelined so the PE never waits
  - relu runs on Scalar (even chunks) and Vector (odd chunks) engines,
    emitting scaled fp8e4 attention weights
  - attn @ v in fp8e4 with DoubleRow perf mode (2x matmul throughput),
    accumulating 256 contraction rows per step in f32 PSUM
"""

import sys

sys.path.insert(0, "/opt/trn_rl_repo")

import numpy as np
import ml_dtypes

C = 64          # channels
NW = 64         # windows (8x8 grid of 32x32 patches on 256x256)
T = 1024        # tokens per window (32*32)
NCORES = 8
WPC = NW // NCORES  # windows per core
BF16 = ml_dtypes.bfloat16

_CACHE = {}


def _build_program(s_a):
    import concourse.mybir as mybir
    from concourse import bacc
    from concourse.tile import TileContext

    bf16 = mybir.dt.bfloat16
    f32 = mybir.dt.float32
    f8 = mybir.dt.float8e4

    nc = bacc.Bacc(None, target_bir_lowering=False)
    # c-major [c, i, t] for q_m/k_m; v pre-packed for DoubleRow:
    # v8[p, i, kk, j, c] = v[i, 256*kk + 128*j + p, c] (scaled to fp8)
    qm_d = nc.declare_dram_parameter("qm", [C, WPC, T], bf16, isOutput=False)
    km_d = nc.declare_dram_parameter("km", [C, WPC, T], bf16, isOutput=False)
    v_d = nc.declare_dram_parameter("v", [128, WPC, 4, 2, C], f8, isOutput=False)
    o_d = nc.declare_dram_parameter("o", [C, WPC, T], bf16, isOutput=True)

    with TileContext(nc) as tc:
        with (
            tc.tile_pool(name="in", bufs=1) as in_pool,
            tc.tile_pool(name="at", bufs=3) as a_pool,
            tc.tile_pool(name="pa", bufs=3, space="PSUM") as pa_pool,
            tc.tile_pool(name="po", bufs=1, space="PSUM") as po_pool,
        ):
            # persistent SBUF tiles, one set per window so each window's
            # compute waits only on its own three DMAs
            # qm/km live duplicated in both partition halves ([128, T] with
            # rows 64:128 == rows 0:64) so QK chunks can alternate between
            # PE row-tiles (0,0) and (64,0) and stream concurrently
            qm_w, km_w, v_w, o_w = [], [], [], []
            for i in range(WPC):
                qm_i = in_pool.tile([128, T], bf16, tag=f"qm{i}", name=f"qm{i}")
                km_i = in_pool.tile([128, T], bf16, tag=f"km{i}", name=f"km{i}")
                v_i = in_pool.tile([128, 4, 2, C], f8, tag=f"v{i}", name=f"v{i}")
                o_i = in_pool.tile([C, T], bf16, tag=f"o{i}", name=f"o{i}")
                # inputs issue from the (otherwise idle) GpSimd queue so the
                # SP queue only carries output DMAs; spread window 0 across
                # both queues so its first matmul can start sooner
                eng = [nc.gpsimd, nc.sync] if i == 0 else [nc.gpsimd, nc.gpsimd]
                eng[0].dma_start(out=qm_i[0:64], in_=qm_d[:, i])
                eng[1].dma_start(out=qm_i[64:128], in_=qm_d[:, i])
                eng[0].dma_start(out=km_i[0:64], in_=km_d[:, i])
                eng[1].dma_start(out=km_i[64:128], in_=km_d[:, i])
                eng[i == 0].dma_start(out=v_i, in_=v_d[:, i])
                qm_w.append(qm_i); km_w.append(km_i)
                v_w.append(v_i); o_w.append(o_i)

            for i in range(WPC):
                # pipelined per window: QK/relu for chunk pair kk+1 issue
                # before the fp8 DoubleRow AV for pair kk
                ps_o = po_pool.tile([C, T], f32, tag="pso")
                at_live = {}

                def emit_pair(kk):
                    # h-major fp8 attn tile: at8[:, h, j, :] so the DoubleRow
                    # rhs slice at8[:, h] is contiguous per partition, and
                    # relu runs at quarter granularity for finer pipelining.
                    # The two chunks of the pair alternate PE row-tiles
                    # ((0,0) for even, (64,0) for odd) and the emission is
                    # h-interleaved, so consecutive matmuls hit disjoint row
                    # groups and stream concurrently in the array.
                    at8 = a_pool.tile([128, 2, 2, 512], f8, tag="attn", name="attn")
                    ps_a = [
                        pa_pool.tile([128, T], f32, tag="psa", name=f"psa{j}")
                        for j in range(2)
                    ]
                    for h in range(2):
                        for j in range(2):
                            k = 2 * kk + j
                            row = 64 * j
                            nc.tensor.matmul(
                                out=ps_a[j][:, h * 512:(h + 1) * 512],
                                lhsT=km_w[i][row:row + 64, k * 128:(k + 1) * 128],
                                rhs=qm_w[i][row:row + 64, h * 512:(h + 1) * 512],
                                tile_position=(row, 0),
                                start=True,
                                stop=True,
                            )
                    for h in range(2):
                        nc.scalar.activation(
                            out=at8[:, h, 0, :],
                            in_=ps_a[0][:, h * 512:(h + 1) * 512],
                            func=mybir.ActivationFunctionType.Relu,
                            scale=float(s_a),
                        )
                        nc.vector.tensor_scalar(
                            out=at8[:, h, 1, :],
                            in0=ps_a[1][:, h * 512:(h + 1) * 512],
                            scalar1=float(s_a),
                            scalar2=0.0,
                            op0=mybir.AluOpType.mult,
                            op1=mybir.AluOpType.max,
                        )
                    at_live[kk] = at8

                emit_pair(0)
                for kk in range(4):
                    if kk + 1 < 4:
                        emit_pair(kk + 1)
                    at8 = at_live.pop(kk)
                    for h in range(2):
                        nc.tensor.matmul(
                            out=ps_o[:, h * 512:(h + 1) * 512],
                            lhsT=v_w[i][:, kk],
                            rhs=at8[:, h],
                            perf_mode=mybir.MatmulPerfMode.DoubleRow,
                            start=(kk == 0),
                            stop=(kk == 3),
                        )
                # split PSUM->SBUF output copy across both free engines
                nc.scalar.activation(
                    out=o_w[i][:, 0:512],
                    in_=ps_o[:, 0:512],
                    func=mybir.ActivationFunctionType.Copy,
                    scale=1.0,
                )
                nc.vector.tensor_copy(out=o_w[i][:, 512:1024], in_=ps_o[:, 512:1024])
                nc.sync.dma_start(out=o_d[:, i], in_=o_w[i])

    nc.finalize()
    return nc


def kernel(x, W, bias, _trace=False):
    global LAST_RESULT
    from concourse.bass_utils import run_bass_kernel_spmd
    import concourse.mybir as mybir

    E4 = mybir.dt.np(mybir.dt.float8e4)

    x = np.asarray(x, dtype=np.float32)
    W = np.asarray(W, dtype=np.float32)
    bias = np.asarray(bias, dtype=np.float32)

    # ---- host prep: windows, qkv, routing, mixing (tiny vs attention) ----
    xw = (
        x.reshape(C, 8, 32, 8, 32)
        .transpose(1, 3, 2, 4, 0)
        .reshape(NW, T, C)
    )
    qkv = xw @ W.T + bias  # [nw, T, 3c]
    q, k, v = qkv[..., :C], qkv[..., C:2 * C], qkv[..., 2 * C:]
    q_r = q.mean(axis=1)  # [nw, c]
    k_r = k.mean(axis=1)
    a_r = np.maximum(q_r @ k_r.T, 0.0)  # [nw, nw]
    k_m = np.tensordot(a_r, k, axes=(1, 0))  # [nw, T, c]
    q_m = np.tensordot(a_r, q, axes=(1, 0))

    # fp8 scales: bound attn logits via Cauchy-Schwarz, v by its max
    bound = max(
        np.linalg.norm(q_m[i], axis=-1).max() * np.linalg.norm(k_m[i], axis=-1).max()
        for i in range(NW)
    )
    s_a = 240.0 / float(bound)
    s_v = 240.0 / float(np.abs(v).max())

    key = ("nc", round(float(s_a), 6))
    if key not in _CACHE:
        _CACHE.clear()
        _CACHE[key] = _build_program(s_a)
    nc = _CACHE[key]

    in_maps = []
    for m in range(NCORES):
        s = slice(m * WPC, (m + 1) * WPC)
        # v8[p, i, kk, j, c] = v[i, 256*kk + 128*j + p, c] * s_v
        v8 = (v[s].reshape(WPC, 4, 2, 128, C) * s_v).astype(E4).transpose(3, 0, 1, 2, 4)
        in_maps.append({
            "qm": np.ascontiguousarray(q_m[s].transpose(2, 0, 1)).astype(BF16),
            "km": np.ascontiguousarray(k_m[s].transpose(2, 0, 1)).astype(BF16),
            "v": np.ascontiguousarray(v8),
        })

    res = run_bass_kernel_spmd(nc, in_maps, list(range(NCORES)), trace=_trace)
    LAST_RESULT = res
    inv = 1.0 / (s_a * s_v)
    outs = [
        res.results[m]["o"].astype(np.float32).reshape(C, WPC, T) * inv
        for m in range(NCORES)
    ]
    o_cm = np.concatenate(outs, axis=1)  # [c, nw, T]

    # fold back: [c, jh, jw, th, tw] -> [1, c, 256, 256]
    o_img = (
        o_cm.reshape(C, 8, 8, 32, 32)
        .transpose(0, 1, 3, 2, 4)
        .reshape(1, C, 256, 256)
    )
    return o_img.astype(np.float32)


LAST_RESULT = None  # BassKernelResults from the most recent run (for test.py)
